# revision 13
# baseline (speedup 1.0000x reference)
"""ArcFace logits kernel for 8 TRN2 NeuronCores (class-parallel / Partial-FC style).

Full computation:
    en = l2norm_rows(embeddings)           # [B, E]
    wn = l2norm_cols(w)                    # [E, C]
    cos = clip(en @ wn, -1+1e-6, 1-1e-6)   # [B, C]
    logits = 64 * where(onehot(labels), margin(cos), cos)

Distribution: class dim C=100000 sharded 12500-per-core (padded to 12544 =
98*128). Embeddings replicated. Each core computes its logits shard
transposed ([C_shard, B]: per-column norm scale is a per-partition scalar).

v6 = the v1 skeleton (per-k w-tile DMAs, 2-tile output stores on the SP
queue, per-tile ACT/DVE drains, fp8 squared-weight column norms) with the
HW-verified wins from v4/v5 ported in:
- host ships raw TRANSPOSED embeddings embT [E,B] bf16 (pure formatting);
  no on-device transposes or PSUM copies. Row norms are computed FROM embT
  (fp16 squares + ones-stationary column-sum matmuls into a [1,B] PSUM row),
  inverted, partition-broadcast by a K=1 matmul into rnB [128,B], and the
  normalized moving operand eTn = embT * rnB is built by 4 DVE multiplies.
  Chunk 0 streams the RAW embT (matmuls start ~2.5us in) and folds the row
  norm into its (deferred) drains; later chunks stream eTn.
- chunk 0's colnorm matmuls + scale run AFTER its dense matmuls (drains
  deferred behind the 6-deep PSUM ring); chunk c>=1's tiny colnorm matmuls
  are hosted 1:1 behind the PREVIOUS chunk's dense matmuls so their weight
  loads hide under 213ns dense streams and each chunk's scale vector is
  ready before its first drain.
- margin path: elementwise prod_k = wlab_k * embT_k (fp16), column-summed
  by ones-stationary matmuls into [1,B] PSUM rows, rearranged into [128,4]
  via a DRAM round-trip (a direct SBUF-row-rearrange DMA miscompiles on HW),
  then the usual margin math. ~3x less PE time than v1's margin.
- an up-front dummy Sqrt pins both ACT table loads to kernel start.

dtype: matmuls bf16 with f32 PSUM accumulation; logits bf16. Dense clip
skipped (|cos| << 1-1e-6 for this distribution; the margin path applies
clip exactly). End-to-end rel err vs the f32 reference ~3.6e-3 (tol 2e-2).
"""

import math
import os
from contextlib import ExitStack

import ml_dtypes
import numpy as np

import concourse.bass as bass
import concourse.tile as tile
from concourse import bacc, mybir
from concourse.bass import ts
from concourse.bass_utils import run_bass_kernel_spmd

ABLATE = set(os.environ.get("ABLATE6", "").split(","))

F32 = mybir.dt.float32
BF16 = mybir.dt.bfloat16
FP16 = mybir.dt.float16
F8 = mybir.dt.float8e4
AF = mybir.ActivationFunctionType
ALU = mybir.AluOpType

B = 512          # batch
E = 512          # embedding dim
C = 100000       # classes
NCORES = 8
CSH = C // NCORES          # 12500 real shard width
CSP = 12544                # padded shard width = 98 * 128
NT = CSP // 128            # 98 C-tiles of 128
NK = E // 128              # 4 contraction blocks

CHUNKS = [6, 10, 14, 14, 14, 14, 14, 12]
assert sum(CHUNKS) == NT
NCH = len(CHUNKS)
CSTART = [0]
for t in CHUNKS:
    CSTART.append(CSTART[-1] + t)
# tiny-MM host window: chunk c's tiny units run in the last W tiles of
# chunk c-1 (None = batched in chunk 0's deferred block)
HOST_WINDOW = {1: None, 2: 6, 3: 10}

MARGIN_C = 3     # emit margin block inside this chunk, after this tile
MARGIN_J = 7

MARGIN = 0.5
SCALE = 64.0
COS_M = math.cos(MARGIN)
SIN_M = math.sin(MARGIN)
TH = math.cos(math.pi - MARGIN)
MM = math.sin(MARGIN) * MARGIN
CLIP_EPS = 1e-6
NORM_EPS = 1e-12
CN_SCALE = 2 ** 18   # pre-scale for squared weights into fp8e4m3 range


def _make_pools(ctx, tc):
    p = {}
    p["sm"] = ctx.enter_context(tc.tile_pool(name="sm", bufs=1))
    p["pw"] = ctx.enter_context(tc.tile_pool(name="pw", bufs=1))
    p["pw2"] = ctx.enter_context(tc.tile_pool(name="pw2", bufs=2))
    p["psd"] = ctx.enter_context(tc.tile_pool(name="psd", bufs=2))
    p["pout"] = ctx.enter_context(tc.tile_pool(name="pout", bufs=8))
    p["psm"] = ctx.enter_context(tc.tile_pool(name="psm", bufs=6, space="PSUM"))
    p["psr"] = ctx.enter_context(tc.tile_pool(name="psr", bufs=1, space="PSUM"))
    p["pscn"] = ctx.enter_context(tc.tile_pool(name="pscn", bufs=1, space="PSUM"))
    return p


def _build_graph(p, tc, nc, embT, wsh, wlab, out, mv, rnscr, pfscr):
    p_sm = p["sm"]

    # --- constants ---
    ones16 = p_sm.tile([128, 1], FP16)
    nc.vector.memset(ones16[:], 1.0)
    ones8 = p_sm.tile([128, 1], F8)
    nc.vector.memset(ones8[:], 1.0)
    onesB = p_sm.tile([1, 128], BF16)
    nc.vector.memset(onesB[:], 1.0)
    eps1 = p_sm.tile([1, 1], F32)
    nc.vector.memset(eps1[:], NORM_EPS)
    eps128 = p_sm.tile([128, 1], F32)
    nc.vector.memset(eps128[:], NORM_EPS)
    # first ACT instruction is a Sqrt so the table pass loads the combined
    # sqrt table (copy+square+sqrt) at kernel start
    warm = p_sm.tile([1, 1], F32)
    nc.scalar.activation(warm[:], eps1[:], AF.Sqrt)

    # --- eTr load (ACT queue), w chunk DMAs (SP queue, v1-style per-k) ---
    eTr_all = p_sm.tile([128, NK * B], BF16)
    for k in range(NK):
        nc.scalar.dma_start(eTr_all[:, ts(k, B)], embT[ts(k, 128), :])
    eTr = [eTr_all[:, ts(k, B)] for k in range(NK)]

    wch = {}

    def issue_w_dma(c):
        c0, c1 = CSTART[c] * 128, CSTART[c + 1] * 128
        tiles = []
        for k in range(NK):
            w_k = p["pw"].tile([128, c1 - c0], BF16, name=f"w_{c}_{k}")
            nc.sync.dma_start(w_k[:], wsh[ts(k, 128), c0:c1])
            tiles.append(w_k)
        wch[c] = tiles

    for c in (0, 1, 2):
        issue_w_dma(c)

    # --- row norms from embT, broadcast tile, normalized moving operand ---
    esq = p_sm.tile([128, NK * B], FP16)
    nc.vector.scalar_tensor_tensor(
        esq[:, : 2 * B], eTr_all[:, : 2 * B], 1.0, eTr_all[:, : 2 * B],
        op0=ALU.mult, op1=ALU.mult,
    )
    nc.scalar.activation(esq[:, 2 * B :], eTr_all[:, 2 * B :], AF.Square)
    psE = p["psr"].tile([128, B], F32, name="psr")
    for k in range(NK):
        nc.tensor.matmul(psE[0:1, :], ones16[:], esq[:, ts(k, B)],
                         start=(k == 0), stop=(k == NK - 1))
    rowE = p_sm.tile([1, B], F32)
    nc.scalar.activation(rowE[:], psE[0:1, :], AF.Sqrt, bias=eps1[:])
    rowR = p_sm.tile([1, B], F32)
    nc.vector.reciprocal(rowR[:], rowE[:])
    rowRb = p_sm.tile([1, B], BF16)
    nc.vector.tensor_copy(rowRb[:], rowR[:])
    # margin-layout row norms rn[q, m] = rowR[0, m*128+q] via DRAM round-trip
    rn = p_sm.tile([128, NK], F32)
    nc.sync.dma_start(rnscr[:, :], rowR[:])
    nc.sync.dma_start(rn[:], rnscr[:, :].rearrange("o (m q) -> (o q) m", q=128))

    rnB = p_sm.tile([128, B], F32)
    eTn_all = p_sm.tile([128, NK * B], BF16)
    eTn = [eTn_all[:, ts(k, B)] for k in range(NK)]

    def emit_norm_part2():
        psR = p["psr"].tile([128, B], F32, name="psr")
        nc.tensor.matmul(psR[:], onesB[:], rowRb[:], start=True, stop=True)
        nc.vector.tensor_copy(rnB[:], psR[:])
        for k in range(NK):
            nc.vector.tensor_mul(eTn_all[:, ts(k, B)], eTr[k], rnB[:])

    # --- colnorm machinery (fp8 squared weights, v1-style) ---
    s_dense = p_sm.tile([128, NT], F32)
    w2ch = {}

    def emit_squares(c):
        if "notiny" in ABLATE:
            return
        width = CHUNKS[c] * 128
        tiles = []
        for k in range(NK):
            w2_k = p["pw2"].tile([128, width], F8, name=f"w2_{k}")
            if k < 2:
                nc.vector.scalar_tensor_tensor(
                    w2_k[:], wch[c][k][:], float(CN_SCALE), wch[c][k][:],
                    op0=ALU.mult, op1=ALU.mult,
                )
            else:
                nc.scalar.activation(
                    w2_k[:], wch[c][k][:], AF.Square,
                    scale=float(math.sqrt(CN_SCALE)),
                )
            tiles.append(w2_k)
        w2ch[c] = tiles

    pscn_t = {}

    def emit_tiny_unit(c, jj, k):
        if "notiny" in ABLATE:
            return
        if jj == 0 and k == 0:
            pscn_t[c] = p["pscn"].tile([128, CHUNKS[c]], F32, name="pscn")
        nc.tensor.matmul(
            pscn_t[c][:, jj : jj + 1], w2ch[c][k][:, ts(jj, 128)], ones8[:],
            start=(k == 0), stop=(k == NK - 1),
        )

    def emit_scale(c):
        if "notiny" in ABLATE:
            if c == 0:
                nc.vector.memset(s_dense[:], 1.0)
            return
        ssq = p["psd"].tile([128, CHUNKS[c]], F32, name="ssq")
        nc.scalar.activation(
            ssq[:], pscn_t[c][:], AF.Sqrt,
            scale=1.0 / (CN_SCALE * SCALE * SCALE), bias=eps128[:],
        )
        nc.vector.reciprocal(s_dense[:, CSTART[c] : CSTART[c + 1]], ssq[:])

    def emit_chunk_colnorm(c):
        emit_squares(c)
        for jj in range(CHUNKS[c]):
            for k in range(NK):
                emit_tiny_unit(c, jj, k)
        emit_scale(c)

    # hosted-ahead tiny schedule: chunk c's units spread over the last
    # HOST_WINDOW[c] tiles of chunk c-1 (1 unit after each big matmul)
    def host_plan(c):
        w = HOST_WINDOW.get(c, CHUNKS[c - 1])
        units = [(jj, k) for jj in range(CHUNKS[c]) for k in range(NK)]
        slots = [(j, k) for j in range(CHUNKS[c - 1] - w, CHUNKS[c - 1])
                 for k in range(NK)]
        plan = {}
        n, m = len(units), len(slots)
        for i, u in enumerate(units):
            plan.setdefault(slots[i * m // n], []).append(u)
        return plan

    # --- margin block ---
    wl_all = p_sm.tile([128, NK * B], BF16)

    def load_wlab():
        for k in range(NK):
            nc.scalar.dma_start(wl_all[:, ts(k, B)], wlab[ts(k, 128), :])

    def emit_margin():
        if "nomargin" in ABLATE:
            return
        prod_all = p_sm.tile([128, NK * B], FP16)
        wl2_all = p_sm.tile([128, NK * B], FP16)
        for k in range(NK):
            nc.vector.tensor_mul(
                prod_all[:, ts(k, B)], wl_all[:, ts(k, B)], eTr[k]
            )
            nc.scalar.activation(
                wl2_all[:, ts(k, B)], wl_all[:, ts(k, B)], AF.Square
            )
        psA = p["psr"].tile([128, B], F32, name="psr")
        for k in range(NK):
            nc.tensor.matmul(psA[0:1, :], ones16[:], prod_all[:, ts(k, B)],
                             start=(k == 0), stop=(k == NK - 1))
        rowAB = p_sm.tile([1, 2 * B], F32)
        nc.scalar.activation(rowAB[:, :B], psA[0:1, :], AF.Copy)
        psB = p["psr"].tile([128, B], F32, name="psr")
        for k in range(NK):
            nc.tensor.matmul(psB[0:1, :], ones16[:], wl2_all[:, ts(k, B)],
                             start=(k == 0), stop=(k == NK - 1))
        nc.vector.tensor_copy(rowAB[:, B:], psB[0:1, :])
        nc.sync.dma_start(pfscr[:, :], rowAB[:])
        pf = p_sm.tile([128, 2 * NK], F32)
        nc.sync.dma_start(
            pf[:], pfscr[:, :].rearrange("o (m q) -> (o q) m", q=128)
        )
        psA_r, psB_r = pf[:, :NK], pf[:, NK:]

        swl_s = p_sm.tile([128, NK], F32)
        nc.scalar.activation(swl_s[:], psB_r, AF.Sqrt, bias=eps128[:])
        s_wl = p_sm.tile([128, NK], F32)
        nc.vector.reciprocal(s_wl[:], swl_s[:])
        cosu = p_sm.tile([128, NK], F32)
        nc.vector.tensor_mul(cosu[:], psA_r, s_wl[:])
        cos_lab = p_sm.tile([128, NK], F32)
        nc.vector.tensor_mul(cos_lab[:], cosu[:], rn[:])

        cc = p_sm.tile([128, NK], F32)
        nc.vector.tensor_scalar_min(cc[:], cos_lab[:], 1.0 - CLIP_EPS)
        nc.vector.tensor_scalar_max(cc[:], cc[:], -1.0 + CLIP_EPS)
        c2 = p_sm.tile([128, NK], F32)
        nc.scalar.activation(c2[:], cc[:], AF.Square)
        sinv = p_sm.tile([128, NK], F32)
        nc.scalar.activation(sinv[:], c2[:], AF.Sqrt, scale=-1.0, bias=1.0)
        t1 = p_sm.tile([128, NK], F32)
        nc.vector.tensor_scalar_mul(t1[:], cc[:], COS_M)
        cm = p_sm.tile([128, NK], F32)
        nc.vector.scalar_tensor_tensor(
            cm[:], sinv[:], -SIN_M, t1[:], op0=ALU.mult, op1=ALU.add
        )
        alt = p_sm.tile([128, NK], F32)
        nc.vector.tensor_scalar_sub(alt[:], cc[:], MM)
        mk = p_sm.tile([128, NK], mybir.dt.int32)
        nc.vector.tensor_scalar(mk[:], cc[:], TH, None, op0=ALU.is_gt)
        res = p_sm.tile([128, NK], F32)
        nc.vector.tensor_copy(res[:], alt[:])
        nc.vector.copy_predicated(res[:], mk[:], cm[:])
        mvt = p_sm.tile([128, NK], F32)
        nc.vector.tensor_scalar_mul(mvt[:], res[:], SCALE)
        nc.sync.dma_start(mv[:, :], mvt[:])

    # --- main loop ---
    for c in range(NCH):
        tch = CHUNKS[c]
        if c == 1:
            load_wlab()
        if c >= 1 and c + 2 < NCH:
            issue_w_dma(c + 2)
        if c >= 2:
            # squares for the chunk whose tinies we host (c+1), emitted a
            # little into this chunk so its w DMA has landed
            pass
        moving = eTr if c == 0 else eTn
        plan = host_plan(c + 1) if c + 1 < NCH and c >= 1 else {}
        hosted_sq_done = [False]
        deferred = []
        for j in range(tch):
            t = CSTART[c] + j
            if j % 2 == 0:
                ot = p["pout"].tile([128, 2 * B], BF16, name="ot")
            psm = p["psm"].tile([128, B], F32, name="psm")
            for k in range(NK):
                nc.tensor.matmul(
                    psm[:], wch[c][k][:, ts(j, 128)], moving[k],
                    start=(k == 0), stop=(k == NK - 1),
                )
                units = plan.get((j, k), [])
                if units and not hosted_sq_done[0]:
                    emit_squares(c + 1)
                    hosted_sq_done[0] = True
                for (jj, kk) in units:
                    emit_tiny_unit(c + 1, jj, kk)
                    if jj == CHUNKS[c + 1] - 1 and kk == NK - 1:
                        emit_scale(c + 1)
            if c == 1 and j == 1:
                # chunk 0's colnorms + chunk 1's (hosted here, batched):
                # both run while chunk 1's dense matmuls stream
                pass
            if c == MARGIN_C and j == MARGIN_J:
                emit_margin()

            def drain(j=j, t=t, ot=ot, psm=psm, c=c):
                half = ot[:, ts(j % 2, B)]
                if c == 0:
                    nc.vector.scalar_tensor_tensor(
                        half, psm[:], s_dense[:, t : t + 1], rnB[:],
                        op0=ALU.mult, op1=ALU.mult,
                    )
                elif j % 2 == 0:
                    nc.scalar.activation(half, psm[:], AF.Copy,
                                         scale=s_dense[:, t : t + 1])
                else:
                    nc.vector.tensor_scalar_mul(half, psm[:],
                                                s_dense[:, t : t + 1])

            def store(t=t, ot=ot):
                t0 = t - 1
                dst = out[t0 * 128 : (t0 + 2) * 128, :].rearrange(
                    "(i q) b -> q i b", q=128
                )
                srcv = ot[:].rearrange("q (i b) -> q i b", i=2)
                nc.sync.dma_start(dst, srcv)

            if c == 0:
                deferred.append(drain)
                if j % 2 == 1:
                    deferred.append(store)
            else:
                drain()
                if j % 2 == 1:
                    store()
        if c == 0:
            # norm part 2 (rnB + eTn), chunk 0 + 1 colnorms, then chunk 0's
            # deferred drains/stores
            emit_norm_part2()
            emit_chunk_colnorm(0)
            emit_chunk_colnorm(1)
            for fn in deferred:
                fn()


_NC_CACHE = {}


def _build(reps=1):
    """Build + compile. reps>1 wraps the whole body in a HW loop (for timing)."""
    if reps in _NC_CACHE:
        return _NC_CACHE[reps]
    nc = bacc.Bacc("TRN2", target_bir_lowering=False, debug=False)
    embT = nc.dram_tensor("embT", [E, B], BF16, kind="ExternalInput").ap()
    wsh = nc.dram_tensor("w_shard", [E, CSP], BF16, kind="ExternalInput").ap()
    wlab = nc.dram_tensor("wlab", [E, B], BF16, kind="ExternalInput").ap()
    out = nc.dram_tensor("out", [CSP, B], BF16, kind="ExternalOutput").ap()
    mv = nc.dram_tensor("mvals", [128, NK], F32, kind="ExternalOutput").ap()
    rnscr = nc.dram_tensor("rnscr", [1, B], F32).ap()
    pfscr = nc.dram_tensor("pfscr", [1, 2 * B], F32).ap()
    with tile.TileContext(nc) as tc, ExitStack() as ctx:
        pools = _make_pools(ctx, tc)
        if reps == 1:
            _build_graph(pools, tc, nc, embT, wsh, wlab, out, mv, rnscr, pfscr)
        else:
            hints = (
                mybir.EngineType.PE,
                mybir.EngineType.DVE,
                mybir.EngineType.Activation,
                mybir.EngineType.SP,
            )
            with tc.For_i(0, reps, 1, hint_engines=hints):
                _build_graph(pools, tc, nc, embT, wsh, wlab, out, mv,
                             rnscr, pfscr)
    nc.compile()
    _NC_CACHE[reps] = nc
    return nc


def _prep_inputs(embeddings, labels, w):
    embf = np.asarray(embeddings, dtype=np.float32).astype(ml_dtypes.bfloat16)
    embT = np.ascontiguousarray(embf.T)
    lab = np.asarray(labels).astype(np.int64)
    wf = np.asarray(w, dtype=np.float32)
    wb = wf.astype(ml_dtypes.bfloat16)
    wlab = np.ascontiguousarray(wb[:, lab])
    in_maps = []
    for i in range(NCORES):
        shard = np.zeros((E, CSP), ml_dtypes.bfloat16)
        shard[:, :CSH] = wb[:, i * CSH : (i + 1) * CSH]
        in_maps.append({"embT": embT, "w_shard": shard, "wlab": wlab})
    return lab, in_maps


def _assemble(results, lab):
    out = np.empty((B, C), np.float32)
    for i in range(NCORES):
        out[:, i * CSH : (i + 1) * CSH] = (
            results[i]["out"][:CSH, :].T.astype(np.float32)
        )
    mvals = results[0]["mvals"].T.reshape(B)
    out[np.arange(B), lab] = mvals
    return out


def kernel(embeddings, labels, w):
    nc = _build()
    lab, in_maps = _prep_inputs(embeddings, labels, w)
    r = run_bass_kernel_spmd(nc, in_maps, core_ids=list(range(NCORES)))
    return _assemble(r.results, lab)


def kernel_profiled(embeddings, labels, w, **trace_kwargs):
    """Like kernel() but traces; returns (output, BassKernelResults)."""
    nc = _build()
    lab, in_maps = _prep_inputs(embeddings, labels, w)
    r = run_bass_kernel_spmd(
        nc, in_maps, core_ids=list(range(NCORES)), trace=True, **trace_kwargs
    )
    return _assemble(r.results, lab), r
